# revision 12
# baseline (speedup 1.0000x reference)
"""Trainium2 Bass kernel for nn_CrossModalFusionBlock.

Strategy: data-parallel over batch (B=16 -> 2 batch rows / core on 8 cores).
All on-device GEMMs run in fp32r (fp32 rounded to 11 mantissa bits; full PE
rate at free-dim >= 256). Activations are kept feature-major ([feature, token])
on device so every GEMM contracts along the partition dim; the host does the
(cheap, layout-only) transposes during shard/unshard.

Per-core pipeline (TOK = 4096 tokens):
  A1: m0 = Wp0@x0 + bp0, m1 = Wp1@x1 + bp1, Q = Wq@(sc) + (Wq@query + bq)
  A2: K_m = Wk@m_m, attention over M=3 modalities (scores via block-ones
      matmuls, softmax on DVE/ACT, head-broadcast via mask matmuls),
      V_m = Wv@m_m folded into ctx accumulation; outputs ctx and the
      head-averaged attn weights.
  A3: attn_out = Wo@ctx + bo, LN1 -> fused1
  B:  ff = W2@gelu(W1@fused1 + b1) + b2 in two 2048-wide slices of the 4H dim
      (ff accumulated in PSUM across slices), then LN2(fused1 + ff).
"""
import numpy as np

import concourse.bacc as bacc
import concourse.mybir as mybir
import concourse.tile as tile
from concourse.bass_utils import run_bass_kernel_spmd

P = 128
H = 1024
HT = H // P          # 8 feature tiles
FF = 4096
NH = 16
HD = 64
M = 3
IN0, IN1 = 256, 512
B_FULL, T_FULL = 16, 2048
N_CORES = 8

F32 = mybir.dt.float32
R = mybir.dt.float32r
AF = mybir.ActivationFunctionType


def pack_fp32r(a: np.ndarray) -> np.ndarray:
    """Round fp32 values to fp32r (RNE to 11 explicit mantissa bits)."""
    u = np.ascontiguousarray(a, dtype=np.float32).view(np.uint32)
    drop = np.uint32(12)
    half = np.uint32(1 << 11)
    lsb = (u >> drop) & np.uint32(1)
    r = ((u + half - np.uint32(1) + lsb) >> drop) << drop
    return r.view(np.float32)


def _wtiles(ap, p=P):
    """DRAM [K, N] -> [p, K//p, N] access pattern (k-tiles on partitions)."""
    return ap.rearrange("(kt p) n -> p kt n", p=p)


def build_program(tok: int, debug: bool = False):
    """Build + compile the per-core program for `tok` tokens."""
    CA = 512            # chunk for A1/A3
    CB = 256            # chunk for A2/B
    NCA = tok // CA
    NCB = tok // CB

    nc = bacc.Bacc("TRN2", target_bir_lowering=False)

    # --- external inputs (activations feature-major, fp32r-packed) ---
    x0T = nc.dram_tensor("x0T", [IN0, tok], R, kind="ExternalInput")
    x1T = nc.dram_tensor("x1T", [IN1, tok], R, kind="ExternalInput")
    x2T = nc.dram_tensor("x2T", [H, tok], R, kind="ExternalInput")
    scT = nc.dram_tensor("scT", [H, tok], R, kind="ExternalInput")
    wp0T = nc.dram_tensor("wp0T", [IN0, H], R, kind="ExternalInput")
    wp1T = nc.dram_tensor("wp1T", [IN1, H], R, kind="ExternalInput")
    wqT = nc.dram_tensor("wqT", [H, H], R, kind="ExternalInput")
    wkT = nc.dram_tensor("wkT", [H, H], R, kind="ExternalInput")
    wvT = nc.dram_tensor("wvT", [H, H], R, kind="ExternalInput")
    woT = nc.dram_tensor("woT", [H, H], R, kind="ExternalInput")
    w1T = nc.dram_tensor("w1T", [H, FF], R, kind="ExternalInput")
    w2T = nc.dram_tensor("w2T", [FF, H], R, kind="ExternalInput")
    # biases / LN params, laid out [P, ntiles]
    bias_names = ["bp0", "bp1", "bq2", "bk", "bv", "bo", "b2", "g1", "be1", "g2", "be2"]
    bias_d = {n: nc.dram_tensor(n, [P, HT], F32, kind="ExternalInput") for n in bias_names}
    bias_d["b1"] = nc.dram_tensor("b1", [P, FF // P], F32, kind="ExternalInput")
    # constants
    ones_mean = nc.dram_tensor("ones_mean", [P, 1], R, kind="ExternalInput")   # 1/H
    ones_bc = nc.dram_tensor("ones_bc", [1, P], R, kind="ExternalInput")       # 1.0
    hones = nc.dram_tensor("hones", [P, HT, NH], R, kind="ExternalInput")      # head-sum masks
    emask = nc.dram_tensor("emask", [NH, HT, P], R, kind="ExternalInput")      # head-bcast masks
    ones_hm = nc.dram_tensor("ones_hm", [NH, 1], R, kind="ExternalInput")      # 1/NH

    # --- outputs ---
    fusedT = nc.dram_tensor("fusedT", [H, tok], F32, kind="ExternalOutput")
    wmT = nc.dram_tensor("wmT", [M, tok], F32, kind="ExternalOutput")

    with tile.TileContext(nc) as tc, nc.allow_low_precision(reason="fp32r rounding is intentional (matmul operand format)"):
        with tc.tile_pool(name="dram", bufs=1, space="DRAM") as dram:
            if debug:
                m0T = nc.dram_tensor("d_m0T", [H, tok], R, kind="ExternalOutput").ap()
                m1T = nc.dram_tensor("d_m1T", [H, tok], R, kind="ExternalOutput").ap()
                qT = nc.dram_tensor("d_qT", [H, tok], R, kind="ExternalOutput").ap()
                cxT = nc.dram_tensor("d_cxT", [H, tok], R, kind="ExternalOutput").ap()
                f1T = nc.dram_tensor("d_f1T", [H, tok], R, kind="ExternalOutput").ap()
                ffT = nc.dram_tensor("d_ffT", [H, tok], F32, kind="ExternalOutput").ap()
            else:
                m0T = dram.tile([H, tok], R)
                m1T = dram.tile([H, tok], R)
                qT = dram.tile([H, tok], R)
                cxT = dram.tile([H, tok], R)
                f1T = dram.tile([H, tok], R)
                ffT = dram.tile([H, tok], F32)

            consts = {}
            with tc.tile_pool(name="consts", bufs=1) as cpool:
                for name, dt_, dr in (
                    ("ones_mean", R, ones_mean), ("ones_bc", R, ones_bc),
                    ("hones", R, hones), ("emask", R, emask), ("ones_hm", R, ones_hm),
                ):
                    t = cpool.tile(list(dr.shape), dt_, name=name)
                    nc.sync.dma_start(t, dr.ap())
                    consts[name] = t
                eps_t = cpool.tile([1, 1], F32, name="eps")
                nc.vector.memset(eps_t, 1e-5)
                consts["eps"] = eps_t
                bias = {}
                for n, dr in bias_d.items():
                    t = cpool.tile(list(dr.shape), F32, name=f"b_{n}")
                    nc.sync.dma_start(t, dr.ap())
                    bias[n] = t

                _phase_a1(nc, tc, CA, NCA, x0T, x1T, scT, wp0T, wp1T, wqT,
                          bias, m0T, m1T, qT)
                _phase_a2(nc, tc, CB, NCB, x2T, m0T, m1T, qT, wkT, wvT,
                          bias, consts, cxT, wmT)
                _phase_a3(nc, tc, CA, NCA, cxT, woT, bias, consts, f1T)
                _phase_b(nc, tc, CB, NCB, f1T, w1T, w2T, bias, consts, ffT, fusedT)

    nc.compile()
    return nc


def _gemm(nc, ps_pool, out_tile, w_sb, x_sb, kt, n, bias_ap=None, act=AF.Copy,
          ho_range=None, psum_name=None):
    """out_tile[:, ho] = act(sum_k w_sb[:,k,ho*P:+P].T @ x_sb[:,k] + bias[:,ho])."""
    if ho_range is None:
        ho_range = range(out_tile.shape[1])
    for ho in ho_range:
        ps = ps_pool.tile([P, n], F32, name=psum_name or "gemm_ps")
        for k in range(kt):
            nc.tensor.matmul(ps, w_sb[:, k, ho * P:(ho + 1) * P], x_sb[:, k],
                             start=(k == 0), stop=(k == kt - 1))
        if bias_ap is not None:
            fn = AF.Identity if act == AF.Copy else act
            nc.scalar.activation(out_tile[:, ho], ps, fn, bias=bias_ap[:, ho:ho + 1])
        else:
            nc.scalar.activation(out_tile[:, ho], ps, act)


def _phase_a1(nc, tc, CA, NCA, x0T, x1T, scT, wp0T, wp1T, wqT, bias, m0T, m1T, qT):
    with (
        tc.tile_pool(name="a1_w", bufs=1) as wp,
        tc.tile_pool(name="a1_in", bufs=2) as ip,
        tc.tile_pool(name="a1_out", bufs=2) as op,
        tc.tile_pool(name="a1_ps", bufs=4, space="PSUM") as pp,
    ):
        w0 = wp.tile([P, IN0 // P, H], R, name="wp0")
        w1 = wp.tile([P, IN1 // P, H], R, name="wp1")
        wq = wp.tile([P, HT, H], R, name="wq")
        nc.sync.dma_start(w0, _wtiles(wp0T.ap()))
        nc.sync.dma_start(w1, _wtiles(wp1T.ap()))
        nc.sync.dma_start(wq, _wtiles(wqT.ap()))
        for c in range(NCA):
            sl = slice(c * CA, (c + 1) * CA)
            x0c = ip.tile([P, IN0 // P, CA], R, name="x0c")
            x1c = ip.tile([P, IN1 // P, CA], R, name="x1c")
            scc = ip.tile([P, HT, CA], R, name="scc")
            nc.sync.dma_start(x0c, _wtiles(x0T.ap())[:, :, sl])
            nc.sync.dma_start(x1c, _wtiles(x1T.ap())[:, :, sl])
            nc.sync.dma_start(scc, _wtiles(scT.ap())[:, :, sl])
            for w_sb, xc, kt, bn, outT, nm in (
                (w0, x0c, IN0 // P, "bp0", m0T, "m0"),
                (w1, x1c, IN1 // P, "bp1", m1T, "m1"),
                (wq, scc, HT, "bq2", qT, "q"),
            ):
                o = op.tile([P, HT, CA], R, name="a1o")
                _gemm(nc, pp, o, w_sb, xc, kt, CA, bias[bn])
                nc.sync.dma_start(_wtiles(outT)[:, :, sl], o)


def _phase_a2(nc, tc, CB, NCB, x2T, m0T, m1T, qT, wkT, wvT, bias, consts, cxT, wmT):
    with (
        tc.tile_pool(name="a2_w", bufs=1) as wp,
        tc.tile_pool(name="a2_in", bufs=2) as ip,
        tc.tile_pool(name="a2_kv", bufs=2) as kvp,
        tc.tile_pool(name="a2_vv", bufs=1) as vvp,
        tc.tile_pool(name="a2_kk", bufs=1) as kkp,
        tc.tile_pool(name="a2_sm", bufs=1) as smp,
        tc.tile_pool(name="a2_ctx", bufs=2) as cxp,
        tc.tile_pool(name="a2_ps", bufs=3, space="PSUM") as pp,
        tc.tile_pool(name="a2_ps_sc", bufs=1, space="PSUM") as pp_sc,
        tc.tile_pool(name="a2_ps_bc", bufs=3, space="PSUM") as pp_bc,
    ):
        wk = wp.tile([P, HT, H], R, name="wk")
        wv = wp.tile([P, HT, H], R, name="wv")
        nc.sync.dma_start(wk, _wtiles(wkT.ap()))
        nc.sync.dma_start(wv, _wtiles(wvT.ap()))
        hones, emask, ones_hm = consts["hones"], consts["emask"], consts["ones_hm"]
        for c in range(NCB):
            sl = slice(c * CB, (c + 1) * CB)
            x2c = ip.tile([P, HT, CB], R, name="x2c")
            m0c = ip.tile([P, HT, CB], R, name="m0c")
            m1c = ip.tile([P, HT, CB], R, name="m1c")
            qc = ip.tile([P, HT, CB], R, name="qc")
            nc.sync.dma_start(x2c, _wtiles(x2T.ap())[:, :, sl])
            nc.sync.dma_start(m0c, _wtiles(m0T)[:, :, sl])
            nc.sync.dma_start(m1c, _wtiles(m1T)[:, :, sl])
            nc.sync.dma_start(qc, _wtiles(qT)[:, :, sl])
            srcs = (m0c, m1c, x2c)

            # K GEMMs + scores
            sc_ps = pp_sc.tile([NH, M, CB], F32, name="sc_ps")
            for m in range(M):
                kk = kkp.tile([P, HT, CB], R, name="kk")
                _gemm(nc, pp, kk, wk, srcs[m], HT, CB, bias["bk"])
                pr = kvp.tile([P, HT, CB], R, name="pr")
                for f in range(HT):
                    nc.vector.tensor_mul(pr[:, f], qc[:, f], kk[:, f])
                    nc.tensor.matmul(sc_ps[:, m, :], hones[:, f], pr[:, f],
                                     start=(f == 0), stop=(f == HT - 1))
            # softmax over modalities (scale 1/8 inside exp)
            sc_sb = smp.tile([NH, M, CB], F32, name="sc_sb")
            nc.scalar.activation(sc_sb, sc_ps, AF.Copy)
            mx = smp.tile([NH, CB], F32, name="mx")
            nc.vector.tensor_max(mx, sc_sb[:, 0, :], sc_sb[:, 1, :])
            nc.vector.tensor_max(mx, mx, sc_sb[:, 2, :])
            es = []
            for m in range(M):
                d = smp.tile([NH, CB], F32, name="d")
                nc.vector.tensor_sub(d, sc_sb[:, m, :], mx)
                e = smp.tile([NH, CB], F32, name=f"e{m}")
                nc.scalar.activation(e, d, AF.Exp, scale=0.125)
                es.append(e)
            ssum = smp.tile([NH, CB], F32, name="ssum")
            nc.vector.tensor_add(ssum, es[0], es[1])
            nc.vector.tensor_add(ssum, ssum, es[2])
            rr = smp.tile([NH, CB], F32, name="rr")
            nc.vector.reciprocal(rr, ssum)
            attn = []
            for m in range(M):
                a = smp.tile([NH, CB], R, name=f"attn{m}")
                nc.vector.tensor_mul(a, es[m], rr)
                attn.append(a)
            # head-mean -> modality weights output
            wm_ps = pp_sc.tile([1, M, CB], F32, name="sc_ps")
            for m in range(M):
                nc.tensor.matmul(wm_ps[:, m, :], ones_hm, attn[m], start=True, stop=True)
            wm_sb = smp.tile([1, M, CB], F32, name="wm_sb")
            nc.scalar.activation(wm_sb, wm_ps, AF.Copy)
            nc.sync.dma_start(wmT.ap()[:, sl], wm_sb)

            # V GEMMs folded into ctx accumulation
            ctx = cxp.tile([P, HT, CB], R, name="ctx")
            for m in range(M):
                vv = vvp.tile([P, HT, CB], F32, name="vv")
                _gemm(nc, pp, vv, wv, srcs[m], HT, CB, bias["bv"])
                for f in range(HT):
                    bc = pp_bc.tile([P, CB], F32, name="bc")
                    nc.tensor.matmul(bc, emask[:, f], attn[m], start=True, stop=True)
                    if m == 0:
                        nc.vector.tensor_mul(ctx[:, f], bc, vv[:, f])
                    else:
                        tmp = smp.tile([P, CB], F32, name="ctmp")
                        nc.vector.tensor_mul(tmp, bc, vv[:, f])
                        nc.vector.tensor_add(ctx[:, f], ctx[:, f], tmp)
            nc.sync.dma_start(_wtiles(cxT)[:, :, sl], ctx)


def _layernorm(nc, tc, pools, x_r, n, g_ap, be_ap, out_tile, out_dt_consts):
    """LN over features of feature-major x_r [P, HT, n] (fp32r) -> out_tile.

    pools: (sbuf_small_pool, psum_stat_pool, psum_bc_pool)
    out_dt_consts: (consts dict)
    """
    smp, pp_st, pp_bc = pools
    consts = out_dt_consts
    sq = smp.tile([P, HT, n], R, name="ln_sq")
    for f in range(HT):
        nc.vector.tensor_mul(sq[:, f], x_r[:, f], x_r[:, f])
    st = pp_st.tile([1, 2, n], F32, name="ln_st")
    for f in range(HT):
        nc.tensor.matmul(st[:, 0, :], consts["ones_mean"], x_r[:, f],
                         start=(f == 0), stop=(f == HT - 1))
    for f in range(HT):
        nc.tensor.matmul(st[:, 1, :], consts["ones_mean"], sq[:, f],
                         start=(f == 0), stop=(f == HT - 1))
    mean = smp.tile([1, n], F32, name="ln_mean")
    nc.scalar.activation(mean, st[:, 0, :], AF.Copy)
    m2 = smp.tile([1, n], F32, name="ln_m2")
    nc.vector.tensor_mul(m2, mean, mean)
    var = smp.tile([1, n], F32, name="ln_var")
    nc.vector.tensor_sub(var, st[:, 1, :], m2)
    sd = smp.tile([1, n], F32, name="ln_sd")
    nc.scalar.activation(sd, var, AF.Sqrt, bias=consts["eps"])
    u = smp.tile([1, n], R, name="ln_u")
    nc.vector.reciprocal(u, sd)
    v = smp.tile([1, n], R, name="ln_v")
    nc.vector.tensor_mul(v, mean, u)
    bc = pp_bc.tile([P, 2, n], F32, name="ln_bc")
    nc.tensor.matmul(bc[:, 0, :], consts["ones_bc"], u, start=True, stop=True)
    nc.tensor.matmul(bc[:, 1, :], consts["ones_bc"], v, start=True, stop=True)
    for f in range(HT):
        t1 = smp.tile([P, n], F32, name="ln_t1")
        nc.vector.tensor_mul(t1, x_r[:, f], bc[:, 0, :])
        t2 = smp.tile([P, n], F32, name="ln_t2")
        nc.vector.tensor_sub(t2, t1, bc[:, 1, :])
        nc.scalar.activation(out_tile[:, f], t2, AF.Identity,
                             bias=be_ap[:, f:f + 1], scale=g_ap[:, f:f + 1])


def _phase_a3(nc, tc, CA, NCA, cxT, woT, bias, consts, f1T):
    with (
        tc.tile_pool(name="a3_w", bufs=1) as wp,
        tc.tile_pool(name="a3_in", bufs=2) as ip,
        tc.tile_pool(name="a3_sm", bufs=2) as smp,
        tc.tile_pool(name="a3_out", bufs=2) as op,
        tc.tile_pool(name="a3_ps", bufs=4, space="PSUM") as pp,
        tc.tile_pool(name="a3_ps_st", bufs=1, space="PSUM") as pp_st,
        tc.tile_pool(name="a3_ps_bc", bufs=1, space="PSUM") as pp_bc,
    ):
        wo = wp.tile([P, HT, H], R, name="wo")
        nc.sync.dma_start(wo, _wtiles(woT.ap()))
        for c in range(NCA):
            sl = slice(c * CA, (c + 1) * CA)
            cxc = ip.tile([P, HT, CA], R, name="cxc")
            nc.sync.dma_start(cxc, _wtiles(cxT)[:, :, sl])
            ao = ip.tile([P, HT, CA], R, name="ao")
            _gemm(nc, pp, ao, wo, cxc, HT, CA, bias["bo"])
            f1 = op.tile([P, HT, CA], R, name="f1")
            _layernorm(nc, tc, (smp, pp_st, pp_bc), ao, CA,
                       bias["g1"], bias["be1"], f1, consts)
            nc.sync.dma_start(_wtiles(f1T)[:, :, sl], f1)


def _phase_b(nc, tc, CB, NCB, f1T, w1T, w2T, bias, consts, ffT, fusedT):
    NS = 2
    SW = FF // NS  # 2048
    SWT = SW // P  # 16
    for s in range(NS):
        with (
            tc.tile_pool(name=f"b_w{s}", bufs=1) as wp,
            tc.tile_pool(name=f"b_in{s}", bufs=2) as ip,
            tc.tile_pool(name=f"b_fc{s}", bufs=1) as fcp,
            tc.tile_pool(name=f"b_h1{s}", bufs=1) as hp,
            tc.tile_pool(name=f"b_sm{s}", bufs=1) as smp,
            tc.tile_pool(name=f"b_out{s}", bufs=1) as op,
            tc.tile_pool(name=f"b_ps{s}", bufs=2, space="PSUM") as pp,
            tc.tile_pool(name=f"b_ps_ff{s}", bufs=1, space="PSUM") as pp_ff,
            tc.tile_pool(name=f"b_ps_st{s}", bufs=1, space="PSUM") as pp_st,
            tc.tile_pool(name=f"b_ps_bc{s}", bufs=1, space="PSUM") as pp_bc,
        ):
            w1 = wp.tile([P, HT, SW], R, name="w1s")
            nc.sync.dma_start(w1, _wtiles(w1T.ap())[:, :, s * SW:(s + 1) * SW])
            w2 = wp.tile([P, SWT, H], R, name="w2s")
            nc.sync.dma_start(w2, _wtiles(w2T.ap())[:, s * SWT:(s + 1) * SWT, :])
            for c in range(NCB):
                sl = slice(c * CB, (c + 1) * CB)
                f1c = ip.tile([P, HT, CB], R, name="f1c")
                nc.sync.dma_start(f1c, _wtiles(f1T)[:, :, sl])
                h1 = hp.tile([P, SWT, CB], R, name="h1")
                _gemm(nc, pp, h1, w1, f1c, HT, CB,
                      bias["b1"][:, s * SWT:(s + 1) * SWT], act=AF.Gelu,
                      ho_range=range(SWT))
                ff_ps = pp_ff.tile([P, HT, CB], F32, name="ff_ps")
                for ho in range(HT):
                    for k in range(SWT):
                        nc.tensor.matmul(ff_ps[:, ho, :], w2[:, k, ho * P:(ho + 1) * P],
                                         h1[:, k], start=(k == 0), stop=(k == SWT - 1))
                if s == 0:
                    ffsb = op.tile([P, HT, CB], F32, name="ffsb")
                    for f in range(HT):
                        nc.scalar.activation(ffsb[:, f], ff_ps[:, f], AF.Copy)
                    nc.sync.dma_start(_wtiles(ffT)[:, :, sl], ffsb)
                else:
                    ffc = fcp.tile([P, HT, CB], F32, name="ffc")
                    nc.sync.dma_start(ffc, _wtiles(ffT)[:, :, sl])
                    xln = op.tile([P, HT, CB], R, name="xln")
                    for f in range(HT):
                        t = smp.tile([P, CB], F32, name="bt")
                        nc.scalar.activation(t, ff_ps[:, f], AF.Identity,
                                             bias=bias["b2"][:, f:f + 1])
                        t2 = smp.tile([P, CB], F32, name="bt2")
                        nc.vector.tensor_add(t2, t, ffc[:, f])
                        nc.vector.tensor_add(xln[:, f], t2, f1c[:, f])
                    outt = op.tile([P, HT, CB], F32, name="outt")
                    _layernorm(nc, tc, (smp, pp_st, pp_bc), xln, CB,
                               bias["g2"], bias["be2"], outt, consts)
                    nc.sync.dma_start(_wtiles(fusedT.ap())[:, :, sl], outt)


# ---------------------------------------------------------------------------
# host side
# ---------------------------------------------------------------------------

_PROGRAM_CACHE = {}


def _get_program(tok, debug=False):
    key = (tok, debug)
    if key not in _PROGRAM_CACHE:
        _PROGRAM_CACHE[key] = build_program(tok, debug)
    return _PROGRAM_CACHE[key]


def _bias_tiles(b):
    """[N] -> [P, N//P] with feature f at [f % P, f // P]."""
    n = b.shape[0]
    return np.ascontiguousarray(b.reshape(n // P, P).T.astype(np.float32))


def make_shared_inputs(Wp0, bp0, Wp1, bp1, Wq, bq, Wk, bk, Wv, bv, Wo, bo,
                       query_token, g1, be1, g2, be2, W1, b1, W2, b2):
    sh = {
        "wp0T": pack_fp32r(Wp0.T), "wp1T": pack_fp32r(Wp1.T),
        "wqT": pack_fp32r(Wq.T), "wkT": pack_fp32r(Wk.T),
        "wvT": pack_fp32r(Wv.T), "woT": pack_fp32r(Wo.T),
        "w1T": pack_fp32r(W1.T), "w2T": pack_fp32r(W2.T),
    }
    bq2 = (Wq.astype(np.float64) @ query_token.astype(np.float64) + bq).astype(np.float32)
    for n, v in (("bp0", bp0), ("bp1", bp1), ("bq2", bq2), ("bk", bk), ("bv", bv),
                 ("bo", bo), ("b1", b1), ("b2", b2), ("g1", g1), ("be1", be1),
                 ("g2", g2), ("be2", be2)):
        sh[n] = _bias_tiles(np.asarray(v))
    sh["ones_mean"] = pack_fp32r(np.full((P, 1), 1.0 / H, np.float32))
    sh["ones_bc"] = pack_fp32r(np.ones((1, P), np.float32))
    hones = np.zeros((P, HT, NH), np.float32)
    for f in range(HT):
        hones[0:HD, f, 2 * f] = 1.0
        hones[HD:P, f, 2 * f + 1] = 1.0
    sh["hones"] = pack_fp32r(hones)
    emask = np.zeros((NH, HT, P), np.float32)
    for f in range(HT):
        emask[2 * f, f, 0:HD] = 1.0
        emask[2 * f + 1, f, HD:P] = 1.0
    sh["emask"] = pack_fp32r(emask)
    sh["ones_hm"] = pack_fp32r(np.full((NH, 1), 1.0 / NH, np.float32))
    return sh


def run_cores(x0, x1, x2, static_context, shared, trace=False, debug=False):
    """x* are [B, T, feat] fp32 full arrays; returns (fused, weights)."""
    B, T = x0.shape[0], x0.shape[1]
    tok = (B // N_CORES) * T
    nc = _get_program(tok, debug=debug)
    in_maps = []
    for i in range(N_CORES):
        bs = slice(i * (B // N_CORES), (i + 1) * (B // N_CORES))
        m = dict(shared)
        m["x0T"] = pack_fp32r(x0[bs].reshape(tok, IN0).T)
        m["x1T"] = pack_fp32r(x1[bs].reshape(tok, IN1).T)
        m["x2T"] = pack_fp32r(x2[bs].reshape(tok, H).T)
        m["scT"] = pack_fp32r(static_context[bs].reshape(tok, H).T)
        in_maps.append(m)
    res = run_bass_kernel_spmd(nc, in_maps, core_ids=list(range(N_CORES)),
                               trace=trace)
    fused = np.empty((B, T, H), np.float32)
    weights = np.empty((B, T, M), np.float32)
    nb = B // N_CORES
    for i in range(N_CORES):
        r = res.results[i]
        fused[i * nb:(i + 1) * nb] = r["fusedT"].T.reshape(nb, T, H)
        weights[i * nb:(i + 1) * nb] = r["wmT"].T.reshape(nb, T, M)
    return fused, weights, res


def kernel(x0, x1, x2, static_context, Wp0, bp0, Wp1, bp1, Wq, bq, Wk, bk,
           Wv, bv, Wo, bo, query_token, g1, be1, g2, be2, W1, b1, W2, b2):
    shared = make_shared_inputs(Wp0, bp0, Wp1, bp1, Wq, bq, Wk, bk, Wv, bv,
                                Wo, bo, query_token, g1, be1, g2, be2,
                                W1, b1, W2, b2)
    fused, weights, _ = run_cores(np.asarray(x0), np.asarray(x1), np.asarray(x2),
                                  np.asarray(static_context), shared)
    return fused, weights


# revision 16
# speedup vs baseline: 1.0166x; 1.0166x over previous
"""Trainium2 Bass kernel for nn_CrossModalFusionBlock.

Strategy: data-parallel over batch (B=16 -> 2 batch rows / core on 8 cores).
All on-device GEMMs run in fp32r (fp32 rounded to 11 mantissa bits; full PE
rate at free-dim >= 256). Activations are kept feature-major ([feature, token])
on device so every GEMM contracts along the partition dim; the host does the
(cheap, layout-only) transposes during shard/unshard.

Per-core pipeline (TOK = 4096 tokens):
  A1: m0 = Wp0@x0 + bp0, m1 = Wp1@x1 + bp1, Q = Wq@(sc) + (Wq@query + bq)
  A2: K_m = Wk@m_m, attention over M=3 modalities (scores via block-ones
      matmuls, softmax on DVE/ACT, head-broadcast via mask matmuls),
      V_m = Wv@m_m folded into ctx accumulation; outputs ctx and the
      head-averaged attn weights.
  A3: attn_out = Wo@ctx + bo, LN1 -> fused1
  B:  ff = W2@gelu(W1@fused1 + b1) + b2 in two 2048-wide slices of the 4H dim
      (ff accumulated in PSUM across slices), then LN2(fused1 + ff).
"""
import numpy as np

import concourse.bacc as bacc
import concourse.mybir as mybir
import concourse.tile as tile
from concourse.bass_utils import run_bass_kernel_spmd

P = 128
H = 1024
HT = H // P          # 8 feature tiles
FF = 4096
NH = 16
HD = 64
M = 3
IN0, IN1 = 256, 512
B_FULL, T_FULL = 16, 2048
N_CORES = 8

F32 = mybir.dt.float32
R = mybir.dt.float32r
AF = mybir.ActivationFunctionType

# pool buffer-count knobs (tuned via TimelineSim)
EVICT_DVE = {"b": True, "a2v": False}
BUFS = {"a1_in": 2, "a1_out": 2, "a2_in": 2, "a2_kv": 2, "a2_vv": 2,
        "a2_kk": 1, "a2_ctx": 2, "a3_in": 2, "a3_out": 2, "b_in": 2,
        "b_h1": 1, "b_out": 1, "b_fc": 1}


def pack_fp32r(a: np.ndarray) -> np.ndarray:
    """Round fp32 values to fp32r (RNE to 11 explicit mantissa bits)."""
    u = np.ascontiguousarray(a, dtype=np.float32).view(np.uint32)
    drop = np.uint32(12)
    half = np.uint32(1 << 11)
    lsb = (u >> drop) & np.uint32(1)
    r = ((u + half - np.uint32(1) + lsb) >> drop) << drop
    return r.view(np.float32)


def _wtiles(ap, p=P):
    """DRAM [K, N] -> [p, K//p, N] access pattern (k-tiles on partitions)."""
    return ap.rearrange("(kt p) n -> p kt n", p=p)


def build_program(tok: int, debug: bool = False):
    """Build + compile the per-core program for `tok` tokens."""
    CA = 512            # chunk for A1/A3
    CB = 256            # chunk for A2/B
    NCA = tok // CA
    NCB = tok // CB

    nc = bacc.Bacc("TRN2", target_bir_lowering=False)

    # --- external inputs (activations feature-major, fp32r-packed) ---
    x0T = nc.dram_tensor("x0T", [IN0, tok], R, kind="ExternalInput")
    x1T = nc.dram_tensor("x1T", [IN1, tok], R, kind="ExternalInput")
    x2T = nc.dram_tensor("x2T", [H, tok], R, kind="ExternalInput")
    scT = nc.dram_tensor("scT", [H, tok], R, kind="ExternalInput")
    wp0T = nc.dram_tensor("wp0T", [IN0, H], R, kind="ExternalInput")
    wp1T = nc.dram_tensor("wp1T", [IN1, H], R, kind="ExternalInput")
    wqT = nc.dram_tensor("wqT", [H, H], R, kind="ExternalInput")
    wkT = nc.dram_tensor("wkT", [H, H], R, kind="ExternalInput")
    wvT = nc.dram_tensor("wvT", [H, H], R, kind="ExternalInput")
    woT = nc.dram_tensor("woT", [H, H], R, kind="ExternalInput")
    w1T = nc.dram_tensor("w1T", [H, FF], R, kind="ExternalInput")
    w2T = nc.dram_tensor("w2T", [FF, H], R, kind="ExternalInput")
    # biases / LN params, laid out [P, ntiles]
    bias_names = ["bp0", "bp1", "bq2", "bk", "bv", "bo", "b2", "g1", "be1", "g2", "be2"]
    bias_d = {n: nc.dram_tensor(n, [P, HT], F32, kind="ExternalInput") for n in bias_names}
    bias_d["b1"] = nc.dram_tensor("b1", [P, FF // P], F32, kind="ExternalInput")
    # constants
    ones_mean = nc.dram_tensor("ones_mean", [P, 1], R, kind="ExternalInput")   # 1/H
    ones_bc = nc.dram_tensor("ones_bc", [1, P], R, kind="ExternalInput")       # 1.0
    hones = nc.dram_tensor("hones", [P, HT, NH], R, kind="ExternalInput")      # head-sum masks
    emask = nc.dram_tensor("emask", [NH, HT, P], R, kind="ExternalInput")      # head-bcast masks
    ones_hm = nc.dram_tensor("ones_hm", [NH, 1], R, kind="ExternalInput")      # 1/NH

    # --- outputs ---
    fusedT = nc.dram_tensor("fusedT", [H, tok], F32, kind="ExternalOutput")
    wmT = nc.dram_tensor("wmT", [M, tok], F32, kind="ExternalOutput")

    with tile.TileContext(nc) as tc, nc.allow_low_precision(reason="fp32r rounding is intentional (matmul operand format)"):
        with tc.tile_pool(name="dram", bufs=1, space="DRAM") as dram:
            if debug:
                m0T = nc.dram_tensor("d_m0T", [H, tok], R, kind="ExternalOutput").ap()
                m1T = nc.dram_tensor("d_m1T", [H, tok], R, kind="ExternalOutput").ap()
                qT = nc.dram_tensor("d_qT", [H, tok], R, kind="ExternalOutput").ap()
                cxT = nc.dram_tensor("d_cxT", [H, tok], R, kind="ExternalOutput").ap()
                f1T = nc.dram_tensor("d_f1T", [H, tok], R, kind="ExternalOutput").ap()
                ffT = nc.dram_tensor("d_ffT", [H, tok], F32, kind="ExternalOutput").ap()
            else:
                m0T = dram.tile([H, tok], R)
                m1T = dram.tile([H, tok], R)
                qT = dram.tile([H, tok], R)
                cxT = dram.tile([H, tok], R)
                f1T = dram.tile([H, tok], R)
                ffT = dram.tile([H, tok], F32)

            consts = {}
            with tc.tile_pool(name="consts", bufs=1) as cpool:
                for name, dt_, dr in (
                    ("ones_mean", R, ones_mean), ("ones_bc", R, ones_bc),
                    ("hones", R, hones), ("emask", R, emask), ("ones_hm", R, ones_hm),
                ):
                    t = cpool.tile(list(dr.shape), dt_, name=name)
                    nc.sync.dma_start(t, dr.ap())
                    consts[name] = t
                eps_t = cpool.tile([1, 1], F32, name="eps")
                nc.vector.memset(eps_t, 1e-5)
                consts["eps"] = eps_t
                bias = {}
                for n, dr in bias_d.items():
                    t = cpool.tile(list(dr.shape), F32, name=f"b_{n}")
                    nc.sync.dma_start(t, dr.ap())
                    bias[n] = t

                _phase_a1(nc, tc, CA, NCA, x0T, x1T, scT, wp0T, wp1T, wqT,
                          bias, m0T, m1T, qT)
                _phase_a2(nc, tc, CB, NCB, x2T, m0T, m1T, qT, wkT, wvT,
                          bias, consts, cxT, wmT)
                _phase_a3(nc, tc, CA, NCA, cxT, woT, bias, consts, f1T)
                _phase_b(nc, tc, CB, NCB, f1T, w1T, w2T, bias, consts, ffT, fusedT)

    nc.compile()
    return nc


def _gemm(nc, ps_pool, out_tile, w_sb, x_sb, kt, n, bias_ap=None, act=AF.Copy,
          ho_range=None, psum_name=None):
    """out_tile[:, ho] = act(sum_k w_sb[:,k,ho*P:+P].T @ x_sb[:,k] + bias[:,ho])."""
    if ho_range is None:
        ho_range = range(out_tile.shape[1])
    for ho in ho_range:
        ps = ps_pool.tile([P, n], F32, name=psum_name or "gemm_ps")
        for k in range(kt):
            nc.tensor.matmul(ps, w_sb[:, k, ho * P:(ho + 1) * P], x_sb[:, k],
                             start=(k == 0), stop=(k == kt - 1))
        if bias_ap is not None:
            fn = AF.Identity if act == AF.Copy else act
            nc.scalar.activation(out_tile[:, ho], ps, fn, bias=bias_ap[:, ho:ho + 1])
        else:
            nc.scalar.activation(out_tile[:, ho], ps, act)


def _phase_a1(nc, tc, CA, NCA, x0T, x1T, scT, wp0T, wp1T, wqT, bias, m0T, m1T, qT):
    with (
        tc.tile_pool(name="a1_w", bufs=1) as wp,
        tc.tile_pool(name="a1_in", bufs=BUFS["a1_in"]) as ip,
        tc.tile_pool(name="a1_out", bufs=BUFS["a1_out"]) as op,
        tc.tile_pool(name="a1_ps", bufs=4, space="PSUM") as pp,
    ):
        w0 = wp.tile([P, IN0 // P, H], R, name="wp0")
        w1 = wp.tile([P, IN1 // P, H], R, name="wp1")
        wq = wp.tile([P, HT, H], R, name="wq")
        nc.sync.dma_start(w0, _wtiles(wp0T.ap()))
        nc.sync.dma_start(w1, _wtiles(wp1T.ap()))
        nc.sync.dma_start(wq, _wtiles(wqT.ap()))
        for c in range(NCA):
            sl = slice(c * CA, (c + 1) * CA)
            x0c = ip.tile([P, IN0 // P, CA], R, name="x0c")
            x1c = ip.tile([P, IN1 // P, CA], R, name="x1c")
            scc = ip.tile([P, HT, CA], R, name="scc")
            nc.sync.dma_start(x0c, _wtiles(x0T.ap())[:, :, sl])
            nc.sync.dma_start(x1c, _wtiles(x1T.ap())[:, :, sl])
            nc.sync.dma_start(scc, _wtiles(scT.ap())[:, :, sl])
            for w_sb, xc, kt, bn, outT, nm in (
                (w0, x0c, IN0 // P, "bp0", m0T, "m0"),
                (w1, x1c, IN1 // P, "bp1", m1T, "m1"),
                (wq, scc, HT, "bq2", qT, "q"),
            ):
                o = op.tile([P, HT, CA], R, name="a1o")
                _gemm(nc, pp, o, w_sb, xc, kt, CA, bias[bn])
                nc.sync.dma_start(_wtiles(outT)[:, :, sl], o)


def _phase_a2(nc, tc, CB, NCB, x2T, m0T, m1T, qT, wkT, wvT, bias, consts, cxT, wmT):
    with (
        tc.tile_pool(name="a2_w", bufs=1) as wp,
        tc.tile_pool(name="a2_in", bufs=BUFS["a2_in"]) as ip,
        tc.tile_pool(name="a2_kv", bufs=BUFS["a2_kv"]) as kvp,
        tc.tile_pool(name="a2_vv", bufs=BUFS["a2_vv"]) as vvp,
        tc.tile_pool(name="a2_kk", bufs=BUFS["a2_kk"]) as kkp,
        tc.tile_pool(name="a2_sm", bufs=1) as smp,
        tc.tile_pool(name="a2_ctx", bufs=BUFS["a2_ctx"]) as cxp,
        tc.tile_pool(name="a2_ps", bufs=BUFS.get("a2_ps",3), space="PSUM") as pp,
        tc.tile_pool(name="a2_ps_sc", bufs=1, space="PSUM") as pp_sc,
        tc.tile_pool(name="a2_ps_bc", bufs=BUFS.get("a2_ps_bc",3), space="PSUM") as pp_bc,
    ):
        wk = wp.tile([P, HT, H], R, name="wk")
        wv = wp.tile([P, HT, H], R, name="wv")
        nc.sync.dma_start(wk, _wtiles(wkT.ap()))
        nc.sync.dma_start(wv, _wtiles(wvT.ap()))
        hones, emask, ones_hm = consts["hones"], consts["emask"], consts["ones_hm"]
        for c in range(NCB):
            sl = slice(c * CB, (c + 1) * CB)
            x2c = ip.tile([P, HT, CB], R, name="x2c")
            m0c = ip.tile([P, HT, CB], R, name="m0c")
            m1c = ip.tile([P, HT, CB], R, name="m1c")
            qc = ip.tile([P, HT, CB], R, name="qc")
            nc.sync.dma_start(x2c, _wtiles(x2T.ap())[:, :, sl])
            nc.sync.dma_start(m0c, _wtiles(m0T)[:, :, sl])
            nc.sync.dma_start(m1c, _wtiles(m1T)[:, :, sl])
            nc.sync.dma_start(qc, _wtiles(qT)[:, :, sl])
            srcs = (m0c, m1c, x2c)

            # K GEMMs + scores
            sc_ps = pp_sc.tile([NH, M, CB], F32, name="sc_ps")
            for m in range(M):
                kk = kkp.tile([P, HT, CB], R, name="kk")
                _gemm(nc, pp, kk, wk, srcs[m], HT, CB, bias["bk"])
                pr = kvp.tile([P, HT, CB], R, name="pr")
                for f in range(HT):
                    nc.vector.tensor_mul(pr[:, f], qc[:, f], kk[:, f])
                    nc.tensor.matmul(sc_ps[:, m, :], hones[:, f], pr[:, f],
                                     start=(f == 0), stop=(f == HT - 1))
            # softmax over modalities (scale 1/8 inside exp)
            sc_sb = smp.tile([NH, M, CB], F32, name="sc_sb")
            nc.scalar.activation(sc_sb, sc_ps, AF.Copy)
            mx = smp.tile([NH, CB], F32, name="mx")
            nc.vector.tensor_max(mx, sc_sb[:, 0, :], sc_sb[:, 1, :])
            nc.vector.tensor_max(mx, mx, sc_sb[:, 2, :])
            es = []
            for m in range(M):
                d = smp.tile([NH, CB], F32, name="d")
                nc.vector.tensor_sub(d, sc_sb[:, m, :], mx)
                e = smp.tile([NH, CB], F32, name=f"e{m}")
                nc.scalar.activation(e, d, AF.Exp, scale=0.125)
                es.append(e)
            ssum = smp.tile([NH, CB], F32, name="ssum")
            nc.vector.tensor_add(ssum, es[0], es[1])
            nc.vector.tensor_add(ssum, ssum, es[2])
            rr = smp.tile([NH, CB], F32, name="rr")
            nc.vector.reciprocal(rr, ssum)
            attn = []
            for m in range(M):
                a = smp.tile([NH, CB], R, name=f"attn{m}")
                nc.vector.tensor_mul(a, es[m], rr)
                attn.append(a)
            # head-mean -> modality weights output
            wm_ps = pp_sc.tile([1, M, CB], F32, name="sc_ps")
            for m in range(M):
                nc.tensor.matmul(wm_ps[:, m, :], ones_hm, attn[m], start=True, stop=True)
            wm_sb = smp.tile([1, M, CB], F32, name="wm_sb")
            nc.scalar.activation(wm_sb, wm_ps, AF.Copy)
            nc.sync.dma_start(wmT.ap()[:, sl], wm_sb)

            # V GEMMs folded into ctx accumulation
            ctx = cxp.tile([P, HT, CB], R, name="ctx")
            for m in range(M):
                vv = vvp.tile([P, HT, CB], F32, name="vv")
                _gemm(nc, pp, vv, wv, srcs[m], HT, CB, bias["bv"])
                for f in range(HT):
                    bc = pp_bc.tile([P, CB], F32, name="bc")
                    nc.tensor.matmul(bc, emask[:, f], attn[m], start=True, stop=True)
                    if m == 0:
                        nc.vector.tensor_mul(ctx[:, f], bc, vv[:, f])
                    else:
                        tmp = smp.tile([P, CB], F32, name="ctmp")
                        nc.vector.tensor_mul(tmp, bc, vv[:, f])
                        nc.vector.tensor_add(ctx[:, f], ctx[:, f], tmp)
            nc.sync.dma_start(_wtiles(cxT)[:, :, sl], ctx)


def _layernorm(nc, tc, pools, x_r, n, g_ap, be_ap, out_tile, out_dt_consts):
    """LN over features of feature-major x_r [P, HT, n] (fp32r) -> out_tile.

    pools: (sbuf_small_pool, psum_stat_pool, psum_bc_pool)
    out_dt_consts: (consts dict)
    """
    smp, pp_st, pp_bc = pools
    consts = out_dt_consts
    sq = smp.tile([P, HT, n], R, name="ln_sq")
    for f in range(HT):
        nc.vector.tensor_mul(sq[:, f], x_r[:, f], x_r[:, f])
    st = pp_st.tile([1, 2, n], F32, name="ln_st")
    for f in range(HT):
        nc.tensor.matmul(st[:, 0, :], consts["ones_mean"], x_r[:, f],
                         start=(f == 0), stop=(f == HT - 1))
    for f in range(HT):
        nc.tensor.matmul(st[:, 1, :], consts["ones_mean"], sq[:, f],
                         start=(f == 0), stop=(f == HT - 1))
    mean = smp.tile([1, n], F32, name="ln_mean")
    nc.scalar.activation(mean, st[:, 0, :], AF.Copy)
    m2 = smp.tile([1, n], F32, name="ln_m2")
    nc.vector.tensor_mul(m2, mean, mean)
    var = smp.tile([1, n], F32, name="ln_var")
    nc.vector.tensor_sub(var, st[:, 1, :], m2)
    sd = smp.tile([1, n], F32, name="ln_sd")
    nc.scalar.activation(sd, var, AF.Sqrt, bias=consts["eps"])
    u = smp.tile([1, n], R, name="ln_u")
    nc.vector.reciprocal(u, sd)
    v = smp.tile([1, n], R, name="ln_v")
    nc.vector.tensor_mul(v, mean, u)
    bc = pp_bc.tile([P, 2, n], F32, name="ln_bc")
    nc.tensor.matmul(bc[:, 0, :], consts["ones_bc"], u, start=True, stop=True)
    nc.tensor.matmul(bc[:, 1, :], consts["ones_bc"], v, start=True, stop=True)
    for f in range(HT):
        t1 = smp.tile([P, n], F32, name="ln_t1")
        nc.vector.tensor_mul(t1, x_r[:, f], bc[:, 0, :])
        t2 = smp.tile([P, n], F32, name="ln_t2")
        nc.vector.tensor_sub(t2, t1, bc[:, 1, :])
        nc.scalar.activation(out_tile[:, f], t2, AF.Identity,
                             bias=be_ap[:, f:f + 1], scale=g_ap[:, f:f + 1])


def _phase_a3(nc, tc, CA, NCA, cxT, woT, bias, consts, f1T):
    with (
        tc.tile_pool(name="a3_w", bufs=1) as wp,
        tc.tile_pool(name="a3_in", bufs=BUFS["a3_in"]) as ip,
        tc.tile_pool(name="a3_sm", bufs=2) as smp,
        tc.tile_pool(name="a3_out", bufs=BUFS["a3_out"]) as op,
        tc.tile_pool(name="a3_ps", bufs=4, space="PSUM") as pp,
        tc.tile_pool(name="a3_ps_st", bufs=1, space="PSUM") as pp_st,
        tc.tile_pool(name="a3_ps_bc", bufs=1, space="PSUM") as pp_bc,
    ):
        wo = wp.tile([P, HT, H], R, name="wo")
        nc.sync.dma_start(wo, _wtiles(woT.ap()))
        for c in range(NCA):
            sl = slice(c * CA, (c + 1) * CA)
            cxc = ip.tile([P, HT, CA], R, name="cxc")
            nc.sync.dma_start(cxc, _wtiles(cxT)[:, :, sl])
            ao = ip.tile([P, HT, CA], R, name="ao")
            _gemm(nc, pp, ao, wo, cxc, HT, CA, bias["bo"])
            f1 = op.tile([P, HT, CA], R, name="f1")
            _layernorm(nc, tc, (smp, pp_st, pp_bc), ao, CA,
                       bias["g1"], bias["be1"], f1, consts)
            nc.sync.dma_start(_wtiles(f1T)[:, :, sl], f1)


def _phase_b(nc, tc, CB, NCB, f1T, w1T, w2T, bias, consts, ffT, fusedT):
    NS = 2
    SW = FF // NS  # 2048
    SWT = SW // P  # 16
    for s in range(NS):
        with (
            tc.tile_pool(name=f"b_w{s}", bufs=1) as wp,
            tc.tile_pool(name=f"b_in{s}", bufs=BUFS["b_in"]) as ip,
            tc.tile_pool(name=f"b_fc{s}", bufs=BUFS["b_fc"]) as fcp,
            tc.tile_pool(name=f"b_h1{s}", bufs=BUFS["b_h1"]) as hp,
            tc.tile_pool(name=f"b_sm{s}", bufs=1) as smp,
            tc.tile_pool(name=f"b_out{s}", bufs=BUFS["b_out"]) as op,
            tc.tile_pool(name=f"b_ps{s}", bufs=BUFS.get("b_ps",2), space="PSUM") as pp,
            tc.tile_pool(name=f"b_ps_ff{s}", bufs=1, space="PSUM") as pp_ff,
            tc.tile_pool(name=f"b_ps_st{s}", bufs=1, space="PSUM") as pp_st,
            tc.tile_pool(name=f"b_ps_bc{s}", bufs=1, space="PSUM") as pp_bc,
        ):
            w1 = wp.tile([P, HT, SW], R, name="w1s")
            nc.sync.dma_start(w1, _wtiles(w1T.ap())[:, :, s * SW:(s + 1) * SW])
            w2 = wp.tile([P, SWT, H], R, name="w2s")
            nc.sync.dma_start(w2, _wtiles(w2T.ap())[:, s * SWT:(s + 1) * SWT, :])
            for c in range(NCB):
                sl = slice(c * CB, (c + 1) * CB)
                f1c = ip.tile([P, HT, CB], R, name="f1c")
                nc.sync.dma_start(f1c, _wtiles(f1T)[:, :, sl])
                h1 = hp.tile([P, SWT, CB], R, name="h1")
                _gemm(nc, pp, h1, w1, f1c, HT, CB,
                      bias["b1"][:, s * SWT:(s + 1) * SWT], act=AF.Gelu,
                      ho_range=range(SWT))
                ff_ps = pp_ff.tile([P, HT, CB], F32, name="ff_ps")
                for ho in range(HT):
                    for k in range(SWT):
                        nc.tensor.matmul(ff_ps[:, ho, :], w2[:, k, ho * P:(ho + 1) * P],
                                         h1[:, k], start=(k == 0), stop=(k == SWT - 1))
                if s == 0:
                    ffsb = op.tile([P, HT, CB], F32, name="ffsb")
                    for f in range(HT):
                        if EVICT_DVE["b"]:
                            nc.vector.tensor_copy(ffsb[:, f], ff_ps[:, f])
                        else:
                            nc.scalar.activation(ffsb[:, f], ff_ps[:, f], AF.Copy)
                    nc.sync.dma_start(_wtiles(ffT)[:, :, sl], ffsb)
                else:
                    ffc = fcp.tile([P, HT, CB], F32, name="ffc")
                    nc.sync.dma_start(ffc, _wtiles(ffT)[:, :, sl])
                    xln = op.tile([P, HT, CB], R, name="xln")
                    for f in range(HT):
                        t = smp.tile([P, CB], F32, name="bt")
                        if EVICT_DVE["b"]:
                            nc.vector.tensor_scalar_add(t, ff_ps[:, f],
                                                        bias["b2"][:, f:f + 1])
                        else:
                            nc.scalar.activation(t, ff_ps[:, f], AF.Identity,
                                                 bias=bias["b2"][:, f:f + 1])
                        t2 = smp.tile([P, CB], F32, name="bt2")
                        nc.vector.tensor_add(t2, t, ffc[:, f])
                        nc.vector.tensor_add(xln[:, f], t2, f1c[:, f])
                    outt = op.tile([P, HT, CB], F32, name="outt")
                    _layernorm(nc, tc, (smp, pp_st, pp_bc), xln, CB,
                               bias["g2"], bias["be2"], outt, consts)
                    nc.sync.dma_start(_wtiles(fusedT.ap())[:, :, sl], outt)


# ---------------------------------------------------------------------------
# host side
# ---------------------------------------------------------------------------

_PROGRAM_CACHE = {}


def _get_program(tok, debug=False):
    key = (tok, debug)
    if key not in _PROGRAM_CACHE:
        _PROGRAM_CACHE[key] = build_program(tok, debug)
    return _PROGRAM_CACHE[key]


def _bias_tiles(b):
    """[N] -> [P, N//P] with feature f at [f % P, f // P]."""
    n = b.shape[0]
    return np.ascontiguousarray(b.reshape(n // P, P).T.astype(np.float32))


def make_shared_inputs(Wp0, bp0, Wp1, bp1, Wq, bq, Wk, bk, Wv, bv, Wo, bo,
                       query_token, g1, be1, g2, be2, W1, b1, W2, b2):
    sh = {
        "wp0T": pack_fp32r(Wp0.T), "wp1T": pack_fp32r(Wp1.T),
        "wqT": pack_fp32r(Wq.T), "wkT": pack_fp32r(Wk.T),
        "wvT": pack_fp32r(Wv.T), "woT": pack_fp32r(Wo.T),
        "w1T": pack_fp32r(W1.T), "w2T": pack_fp32r(W2.T),
    }
    bq2 = (Wq.astype(np.float64) @ query_token.astype(np.float64) + bq).astype(np.float32)
    for n, v in (("bp0", bp0), ("bp1", bp1), ("bq2", bq2), ("bk", bk), ("bv", bv),
                 ("bo", bo), ("b1", b1), ("b2", b2), ("g1", g1), ("be1", be1),
                 ("g2", g2), ("be2", be2)):
        sh[n] = _bias_tiles(np.asarray(v))
    sh["ones_mean"] = pack_fp32r(np.full((P, 1), 1.0 / H, np.float32))
    sh["ones_bc"] = pack_fp32r(np.ones((1, P), np.float32))
    hones = np.zeros((P, HT, NH), np.float32)
    for f in range(HT):
        hones[0:HD, f, 2 * f] = 1.0
        hones[HD:P, f, 2 * f + 1] = 1.0
    sh["hones"] = pack_fp32r(hones)
    emask = np.zeros((NH, HT, P), np.float32)
    for f in range(HT):
        emask[2 * f, f, 0:HD] = 1.0
        emask[2 * f + 1, f, HD:P] = 1.0
    sh["emask"] = pack_fp32r(emask)
    sh["ones_hm"] = pack_fp32r(np.full((NH, 1), 1.0 / NH, np.float32))
    return sh


def run_cores(x0, x1, x2, static_context, shared, trace=False, debug=False):
    """x* are [B, T, feat] fp32 full arrays; returns (fused, weights)."""
    B, T = x0.shape[0], x0.shape[1]
    tok = (B // N_CORES) * T
    nc = _get_program(tok, debug=debug)
    in_maps = []
    for i in range(N_CORES):
        bs = slice(i * (B // N_CORES), (i + 1) * (B // N_CORES))
        m = dict(shared)
        m["x0T"] = pack_fp32r(x0[bs].reshape(tok, IN0).T)
        m["x1T"] = pack_fp32r(x1[bs].reshape(tok, IN1).T)
        m["x2T"] = pack_fp32r(x2[bs].reshape(tok, H).T)
        m["scT"] = pack_fp32r(static_context[bs].reshape(tok, H).T)
        in_maps.append(m)
    res = run_bass_kernel_spmd(nc, in_maps, core_ids=list(range(N_CORES)),
                               trace=trace)
    fused = np.empty((B, T, H), np.float32)
    weights = np.empty((B, T, M), np.float32)
    nb = B // N_CORES
    for i in range(N_CORES):
        r = res.results[i]
        fused[i * nb:(i + 1) * nb] = r["fusedT"].T.reshape(nb, T, H)
        weights[i * nb:(i + 1) * nb] = r["wmT"].T.reshape(nb, T, M)
    return fused, weights, res


def kernel(x0, x1, x2, static_context, Wp0, bp0, Wp1, bp1, Wq, bq, Wk, bk,
           Wv, bv, Wo, bo, query_token, g1, be1, g2, be2, W1, b1, W2, b2):
    shared = make_shared_inputs(Wp0, bp0, Wp1, bp1, Wq, bq, Wk, bk, Wv, bv,
                                Wo, bo, query_token, g1, be1, g2, be2,
                                W1, b1, W2, b2)
    fused, weights, _ = run_cores(np.asarray(x0), np.asarray(x1), np.asarray(x2),
                                  np.asarray(static_context), shared)
    return fused, weights
